# revision 9
# baseline (speedup 1.0000x reference)
"""CrossAttentionOutLayer Trainium2 kernel.

Math: reference computes, per batch b:
    q = rna @ Wq.T + bq                [n, h*dk]
    k = prot @ Wk.T + bk               [m, h*dk]
    logits[h] = (q_h*scale + rel_h) @ k_h.T
    out = mean_h logits                [n, m]

The head-mean of per-head inner products collapses into one flat inner
product over the h*dk=512 axis:
    out[i,j] = (scale/H * q[i,:] + rel_flat/H) . k[j,:]
so with Wq2 = (scale/H)*Wq, bq2 = (scale/H)*bq + rel_flat/H:
    out = (rna @ Wq2.T + bq2) @ (prot @ Wk.T + bk).T
Three GEMMs per batch. Data-parallel: batch b -> core b (8 cores).

On-device layout: feature-major ("transposed") activations via DMA x-bar
transpose (bf16), all GEMMs in bf16 with fp32 PSUM accumulation.
DMA issue is split across SP (transposes), ACT (weights), and GPSIMD
(output stores) so the PE's first matmul gates only on the first weight
chunk + first transposed tile.
"""

import os
from contextlib import ExitStack

import numpy as np
import ml_dtypes

# timing experiment: 1 = replace x-bar transposes with plain same-size DMAs
# (results become WRONG; only for isolating the transpose cost)
_NOTRANS = os.environ.get("KERNEL_NOTRANS", "0") == "1"
# 1 = activations shipped feature-major from the host (plain contiguous
# loads on device); 0 = natural layout + on-device x-bar DMA transpose
_HOSTT = os.environ.get("KERNEL_HOSTT", "1") == "1"

import concourse.bass as bass
import concourse.bacc as bacc
import concourse.tile as tile
import concourse.mybir as mybir
from concourse import bass_utils
from concourse.bass import ts

B, N, M = 8, 1024, 1024
DIM2 = 1280            # rna in-features  = 10*128
KIN = 1344             # protein in-features
KINP = 1408            # padded to 11*128
F = 512                # h*dk flat feature dim = 4*128
H, DK = 8, 64
SCALE = DK ** -0.5
NCORES = 8

NQ = DIM2 // 128       # 10 contraction tiles for Q gemm
NK = KINP // 128       # 11 contraction tiles for K gemm
NF = F // 128          # 4 feature tiles
NB = N // 128          # 8 row blocks of output
NMC = M // 512         # 2 column chunks of output

WK_CHUNKS = [4, 4, 3]  # contraction tiles per weight-load DMA
WQ_CHUNKS = [4, 4, 2]

BF16 = mybir.dt.bfloat16
F32 = mybir.dt.float32

_CACHE = {}


def _build_program(reps=1):
    nc = bacc.Bacc(
        "TRN2", target_bir_lowering=False, debug=False, num_devices=NCORES
    )

    if _HOSTT:
        rna_d = nc.dram_tensor("rna", [DIM2, N], BF16, kind="ExternalInput").ap()
        prot_d = nc.dram_tensor("prot", [KINP, M], BF16, kind="ExternalInput").ap()
    else:
        rna_d = nc.dram_tensor("rna", [N, DIM2], BF16, kind="ExternalInput").ap()
        prot_d = nc.dram_tensor("prot", [M, KINP], BF16, kind="ExternalInput").ap()
    wq_d = nc.dram_tensor("wqt", [DIM2, F], BF16, kind="ExternalInput").ap()
    wk_d = nc.dram_tensor("wkt", [KINP, F], BF16, kind="ExternalInput").ap()
    b2_d = nc.dram_tensor("b2", [128, 2 * NF], F32, kind="ExternalInput").ap()
    out_d = nc.dram_tensor("out", [N, M], BF16, kind="ExternalOutput").ap()

    with tile.TileContext(nc) as tc:
        with (
            tc.tile_pool(name="weights", bufs=1) as wpool,
            tc.tile_pool(name="acts", bufs=1) as apool,
            tc.tile_pool(name="qk", bufs=1) as qkpool,
            tc.tile_pool(name="bias", bufs=1) as bpool,
            tc.tile_pool(name="outs", bufs=4) as opool,
            tc.tile_pool(name="psum", bufs=1, space="PSUM") as pspool,
            ExitStack() as loop_ctx,
        ):
            if reps > 1:
                loop_ctx.enter_context(
                    tc.For_i(
                        0, reps, 1, hint_engines=(mybir.EngineType.PE,)
                    )
                )
            # ---- persistent SBUF tensors ----
            # weight chunk tiles: [128, n_ktiles_in_chunk, F]
            wk_c = [
                wpool.tile([128, sz, F], BF16, tag=f"wkc{j}", name=f"wkc{j}")
                for j, sz in enumerate(WK_CHUNKS)
            ]
            wq_c = [
                wpool.tile([128, sz, F], BF16, tag=f"wqc{j}", name=f"wqc{j}")
                for j, sz in enumerate(WQ_CHUNKS)
            ]

            def chunk_slice(chunks, tiles, i):
                """(chunk_tile, local_idx) for global contraction tile i."""
                j = 0
                while i >= chunks[j]:
                    i -= chunks[j]
                    j += 1
                return tiles[j][:, i]

            xk_t = [
                apool.tile([128, M], BF16, tag=f"xk{i}", name=f"xk{i}")
                for i in range(NK)
            ]
            xq_t = [
                apool.tile([128, N], BF16, tag=f"xq{i}", name=f"xq{i}")
                for i in range(NQ)
            ]
            kt_t = [
                qkpool.tile([128, M], BF16, tag=f"kt{f}", name=f"kt{f}")
                for f in range(NF)
            ]
            q2_t = [
                qkpool.tile([128, N], BF16, tag=f"q2{f}", name=f"q2{f}")
                for f in range(NF)
            ]
            b2_t = bpool.tile([128, 2 * NF], F32, tag="b2", name="b2sb")

            # ---- DMA issue plan ----
            # All input DMAs on SP (nc.sync), ordered exactly as the PE
            # consumes them: weight chunk j lands just before the first
            # matmul that needs it, transposes stream in between.
            def load_xpose(dst, src_d, i):
                if _HOSTT:
                    nc.sync.dma_start(dst, src_d[ts(i, 128), :])
                elif _NOTRANS:
                    nc.sync.dma_start(dst, src_d[0:128, 0 : dst.shape[1]])
                else:
                    nc.sync.dma_start(dst, src_d[:, ts(i, 128)], transpose=True)

            def load_wchunk(w_c, w_d, chunks, j):
                off = sum(chunks[:j])
                src = w_d[off * 128 : (off + chunks[j]) * 128, :]
                nc.sync.dma_start(
                    w_c[j], src.rearrange("(t p) f -> p t f", p=128)
                )

            load_wchunk(wk_c, wk_d, WK_CHUNKS, 0)
            for i in range(3):
                load_xpose(xk_t[i], prot_d, i)
            load_wchunk(wk_c, wk_d, WK_CHUNKS, 1)
            for i in range(3, 6):
                load_xpose(xk_t[i], prot_d, i)
            load_wchunk(wk_c, wk_d, WK_CHUNKS, 2)
            for i in range(6, NK):
                load_xpose(xk_t[i], prot_d, i)
            nc.sync.dma_start(b2_t, b2_d)
            for i in range(3):
                load_xpose(xq_t[i], rna_d, i)
            load_wchunk(wq_c, wq_d, WQ_CHUNKS, 0)
            for i in range(3, 6):
                load_xpose(xq_t[i], rna_d, i)
            load_wchunk(wq_c, wq_d, WQ_CHUNKS, 1)
            for i in range(6, NQ):
                load_xpose(xq_t[i], rna_d, i)
            load_wchunk(wq_c, wq_d, WQ_CHUNKS, 2)

            # ---- GEMM2: kT[f,m] = sum_i WkT[i,f].T @ protT[i,m]  (+bk) ----
            # contraction-outer so PE work starts as soon as xk[0] lands
            ps_k = [
                pspool.tile([128, 512], F32, tag=f"ps{j}", name=f"psk{j}")
                for j in range(8)
            ]
            for i in range(NK):
                wki = chunk_slice(WK_CHUNKS, wk_c, i)
                for f in range(NF):
                    for mc in range(NMC):
                        nc.tensor.matmul(
                            ps_k[f * NMC + mc],
                            wki[:, ts(f, 128)],
                            xk_t[i][:, ts(mc, 512)],
                            start=(i == 0),
                            stop=(i == NK - 1),
                        )
            for f in range(NF):
                for mc in range(NMC):
                    nc.vector.tensor_scalar_add(
                        kt_t[f][:, ts(mc, 512)],
                        ps_k[f * NMC + mc],
                        b2_t[:, f : f + 1],
                    )

            # ---- GEMM1: q2T[f,n] = sum_i WqT[i,f].T @ rnaT[i,n]  (+bq2) ----
            ps_q = [
                pspool.tile([128, 512], F32, tag=f"ps{j}", name=f"psq{j}")
                for j in range(8)
            ]
            for i in range(NQ):
                wqi = chunk_slice(WQ_CHUNKS, wq_c, i)
                for f in range(NF):
                    for nc_ in range(NMC):
                        nc.tensor.matmul(
                            ps_q[f * NMC + nc_],
                            wqi[:, ts(f, 128)],
                            xq_t[i][:, ts(nc_, 512)],
                            start=(i == 0),
                            stop=(i == NQ - 1),
                        )
            for f in range(NF):
                for nc_ in range(NMC):
                    nc.vector.tensor_scalar_add(
                        q2_t[f][:, ts(nc_, 512)],
                        ps_q[f * NMC + nc_],
                        b2_t[:, NF + f : NF + f + 1],
                    )

            # ---- GEMM3: out[n,m] = sum_f q2T[f,n].T @ kT[f,m] ----
            for nb in range(NB):
                for mc in range(NMC):
                    ps = pspool.tile(
                        [128, 512],
                        F32,
                        tag=f"ps{(nb * NMC + mc) % 8}",
                        name=f"ps3_{nb}_{mc}",
                    )
                    for f in range(NF):
                        nc.tensor.matmul(
                            ps,
                            q2_t[f][:, ts(nb, 128)],
                            kt_t[f][:, ts(mc, 512)],
                            start=(f == 0),
                            stop=(f == NF - 1),
                        )
                    ot = opool.tile(
                        [128, 512], BF16, tag="ot", name=f"ot{nb}_{mc}"
                    )
                    if (nb + mc) % 2 == 0:
                        nc.vector.tensor_copy(ot, ps)
                    else:
                        nc.scalar.activation(
                            ot, ps, mybir.ActivationFunctionType.Copy
                        )
                    # stores on SWDGE (Pool) - third parallel DMA path
                    nc.gpsimd.dma_start(out_d[ts(nb, 128), ts(mc, 512)], ot)

    nc.compile()
    return nc


def _get_program(reps=1):
    key = ("nc", reps)
    if key not in _CACHE:
        _CACHE[key] = _build_program(reps)
    return _CACHE[key]


def _prep_inputs(rna_reps, protein_reps, Wq, bq, Wk, bk, rel_bias):
    bf16 = ml_dtypes.bfloat16
    # fold scale/H into Wq; fold rel_bias head-mean into the q bias
    rel_flat = np.asarray(rel_bias, np.float32).reshape(H * DK)
    wq2t = (np.asarray(Wq, np.float32).T * (SCALE / H)).astype(bf16)  # [DIM2,F]
    bq2 = (SCALE / H) * np.asarray(bq, np.float32) + rel_flat / H
    wkt = np.zeros((KINP, F), dtype=bf16)
    wkt[:KIN] = np.asarray(Wk, np.float32).T.astype(bf16)
    bk2 = np.asarray(bk, np.float32)

    # packed biases: col f -> bk chunk f, col NF+f -> bq chunk f
    b2 = np.empty((128, 2 * NF), np.float32)
    for f in range(NF):
        b2[:, f] = bk2[f * 128 : (f + 1) * 128]
        b2[:, NF + f] = bq2[f * 128 : (f + 1) * 128]

    if _HOSTT:
        # feature-major layout: [B, D, tokens]
        rna_bf = (
            np.asarray(rna_reps, np.float32)
            .transpose(0, 2, 1)
            .astype(bf16)
        )
        prot_bf = np.zeros((B, KINP, M), dtype=bf16)
        prot_bf[:, :KIN] = (
            np.asarray(protein_reps, np.float32)
            .transpose(0, 2, 1)
            .astype(bf16)
        )
    else:
        rna_bf = np.asarray(rna_reps, np.float32).astype(bf16)  # [B,N,DIM2]
        prot_bf = np.zeros((B, M, KINP), dtype=bf16)
        prot_bf[:, :, :KIN] = np.asarray(protein_reps, np.float32).astype(bf16)

    in_maps = []
    for b in range(B):
        in_maps.append(
            {
                "rna": np.ascontiguousarray(rna_bf[b]),
                "prot": np.ascontiguousarray(prot_bf[b]),
                "wqt": wq2t,
                "wkt": wkt,
                "b2": b2,
            }
        )
    return in_maps


def kernel(rna_reps, protein_reps, Wq, bq, Wk, bk, rel_bias, **_ignored):
    in_maps = _prep_inputs(rna_reps, protein_reps, Wq, bq, Wk, bk, rel_bias)
    nc = _get_program()
    res = bass_utils.run_bass_kernel_spmd(
        nc, in_maps, core_ids=list(range(NCORES))
    )
    out = np.stack(
        [np.asarray(res.results[b]["out"], np.float32) for b in range(B)], axis=0
    )
    return out


# revision 10
# speedup vs baseline: 1.0838x; 1.0838x over previous
"""CrossAttentionOutLayer Trainium2 kernel.

Math: reference computes, per batch b:
    q = rna @ Wq.T + bq                [n, h*dk]
    k = prot @ Wk.T + bk               [m, h*dk]
    logits[h] = (q_h*scale + rel_h) @ k_h.T
    out = mean_h logits                [n, m]

The head-mean of per-head inner products collapses into one flat inner
product over the h*dk=512 axis:
    out[i,j] = (scale/H * q[i,:] + rel_flat/H) . k[j,:]
so with Wq2 = (scale/H)*Wq, bq2 = (scale/H)*bq + rel_flat/H:
    out = (rna @ Wq2.T + bq2) @ (prot @ Wk.T + bk).T
Three GEMMs per batch. Data-parallel: batch b -> core b (8 cores).

On-device layout: feature-major ("transposed") activations via DMA x-bar
transpose (bf16), all GEMMs in bf16 with fp32 PSUM accumulation.
DMA issue is split across SP (transposes), ACT (weights), and GPSIMD
(output stores) so the PE's first matmul gates only on the first weight
chunk + first transposed tile.
"""

import os
from contextlib import ExitStack

import numpy as np
import ml_dtypes

# timing experiment: 1 = replace x-bar transposes with plain same-size DMAs
# (results become WRONG; only for isolating the transpose cost)
_NOTRANS = os.environ.get("KERNEL_NOTRANS", "0") == "1"
# 1 = activations shipped feature-major from the host (plain contiguous
# loads on device); 0 = natural layout + on-device x-bar DMA transpose
_HOSTT = os.environ.get("KERNEL_HOSTT", "1") == "1"
# 1 = (timing probe) repeat-loop wraps only the matmul/bias phases; DMAs
# and output copies/stores run once outside the loop
_LOOPMM = os.environ.get("KERNEL_LOOPMM", "0") == "1"

import concourse.bass as bass
import concourse.bacc as bacc
import concourse.tile as tile
import concourse.mybir as mybir
from concourse import bass_utils
from concourse.bass import ts

B, N, M = 8, 1024, 1024
DIM2 = 1280            # rna in-features  = 10*128
KIN = 1344             # protein in-features
KINP = 1408            # padded to 11*128
F = 512                # h*dk flat feature dim = 4*128
H, DK = 8, 64
SCALE = DK ** -0.5
NCORES = 8

NQ = DIM2 // 128       # 10 contraction tiles for Q gemm
NK = KINP // 128       # 11 contraction tiles for K gemm
NF = F // 128          # 4 feature tiles
NB = N // 128          # 8 row blocks of output
NMC = M // 512         # 2 column chunks of output

WK_CHUNKS = [4, 4, 3]  # contraction tiles per weight-load DMA
WQ_CHUNKS = [4, 4, 2]

BF16 = mybir.dt.bfloat16
F32 = mybir.dt.float32

_CACHE = {}


def _build_program(reps=1):
    nc = bacc.Bacc(
        "TRN2", target_bir_lowering=False, debug=False, num_devices=NCORES
    )

    if _HOSTT:
        rna_d = nc.dram_tensor("rna", [DIM2, N], BF16, kind="ExternalInput").ap()
        prot_d = nc.dram_tensor("prot", [KINP, M], BF16, kind="ExternalInput").ap()
    else:
        rna_d = nc.dram_tensor("rna", [N, DIM2], BF16, kind="ExternalInput").ap()
        prot_d = nc.dram_tensor("prot", [M, KINP], BF16, kind="ExternalInput").ap()
    wq_d = nc.dram_tensor("wqt", [DIM2, F], BF16, kind="ExternalInput").ap()
    wk_d = nc.dram_tensor("wkt", [KINP, F], BF16, kind="ExternalInput").ap()
    b2_d = nc.dram_tensor("b2", [128, 2 * NF], F32, kind="ExternalInput").ap()
    out_d = nc.dram_tensor("out", [N, M], BF16, kind="ExternalOutput").ap()

    with tile.TileContext(nc) as tc:
        with (
            tc.tile_pool(name="weights", bufs=1) as wpool,
            tc.tile_pool(name="acts", bufs=1) as apool,
            tc.tile_pool(name="qk", bufs=1) as qkpool,
            tc.tile_pool(name="bias", bufs=1) as bpool,
            tc.tile_pool(name="outs", bufs=4) as opool,
            tc.tile_pool(name="psum", bufs=1, space="PSUM") as pspool,
            ExitStack() as loop_ctx,
        ):
            if reps > 1 and not _LOOPMM:
                loop_ctx.enter_context(
                    tc.For_i(
                        0, reps, 1, hint_engines=(mybir.EngineType.PE,)
                    )
                )
            # ---- persistent SBUF tensors ----
            # weight chunk tiles: [128, n_ktiles_in_chunk, F]
            wk_c = [
                wpool.tile([128, sz, F], BF16, tag=f"wkc{j}", name=f"wkc{j}")
                for j, sz in enumerate(WK_CHUNKS)
            ]
            wq_c = [
                wpool.tile([128, sz, F], BF16, tag=f"wqc{j}", name=f"wqc{j}")
                for j, sz in enumerate(WQ_CHUNKS)
            ]

            def chunk_slice(chunks, tiles, i):
                """(chunk_tile, local_idx) for global contraction tile i."""
                j = 0
                while i >= chunks[j]:
                    i -= chunks[j]
                    j += 1
                return tiles[j][:, i]

            xk_t = [
                apool.tile([128, M], BF16, tag=f"xk{i}", name=f"xk{i}")
                for i in range(NK)
            ]
            xq_t = [
                apool.tile([128, N], BF16, tag=f"xq{i}", name=f"xq{i}")
                for i in range(NQ)
            ]
            kt_t = [
                qkpool.tile([128, M], BF16, tag=f"kt{f}", name=f"kt{f}")
                for f in range(NF)
            ]
            q2_t = [
                qkpool.tile([128, N], BF16, tag=f"q2{f}", name=f"q2{f}")
                for f in range(NF)
            ]
            b2_t = bpool.tile([128, 2 * NF], F32, tag="b2", name="b2sb")

            # ---- DMA issue plan ----
            # All input DMAs on SP (nc.sync), ordered exactly as the PE
            # consumes them: weight chunk j lands just before the first
            # matmul that needs it, transposes stream in between.
            def load_xpose(dst, src_d, i):
                if _HOSTT:
                    nc.sync.dma_start(dst, src_d[ts(i, 128), :])
                elif _NOTRANS:
                    nc.sync.dma_start(dst, src_d[0:128, 0 : dst.shape[1]])
                else:
                    nc.sync.dma_start(dst, src_d[:, ts(i, 128)], transpose=True)

            def load_wchunk(w_c, w_d, chunks, j):
                off = sum(chunks[:j])
                src = w_d[off * 128 : (off + chunks[j]) * 128, :]
                nc.sync.dma_start(
                    w_c[j], src.rearrange("(t p) f -> p t f", p=128)
                )

            load_wchunk(wk_c, wk_d, WK_CHUNKS, 0)
            for i in range(3):
                load_xpose(xk_t[i], prot_d, i)
            load_wchunk(wk_c, wk_d, WK_CHUNKS, 1)
            for i in range(3, 6):
                load_xpose(xk_t[i], prot_d, i)
            load_wchunk(wk_c, wk_d, WK_CHUNKS, 2)
            for i in range(6, NK):
                load_xpose(xk_t[i], prot_d, i)
            nc.sync.dma_start(b2_t, b2_d)
            for i in range(3):
                load_xpose(xq_t[i], rna_d, i)
            load_wchunk(wq_c, wq_d, WQ_CHUNKS, 0)
            for i in range(3, 6):
                load_xpose(xq_t[i], rna_d, i)
            load_wchunk(wq_c, wq_d, WQ_CHUNKS, 1)
            for i in range(6, NQ):
                load_xpose(xq_t[i], rna_d, i)
            load_wchunk(wq_c, wq_d, WQ_CHUNKS, 2)

            if reps > 1 and _LOOPMM:
                loop_ctx.enter_context(
                    tc.For_i(
                        0, reps, 1, hint_engines=(mybir.EngineType.PE,)
                    )
                )

            # ---- GEMM2: kT[f,m] = sum_i WkT[i,f].T @ protT[i,m]  (+bk) ----
            # contraction-outer so PE work starts as soon as xk[0] lands
            ps_k = [
                pspool.tile([128, 512], F32, tag=f"ps{j}", name=f"psk{j}")
                for j in range(8)
            ]
            for i in range(NK):
                wki = chunk_slice(WK_CHUNKS, wk_c, i)
                for f in range(NF):
                    for mc in range(NMC):
                        nc.tensor.matmul(
                            ps_k[f * NMC + mc],
                            wki[:, ts(f, 128)],
                            xk_t[i][:, ts(mc, 512)],
                            start=(i == 0),
                            stop=(i == NK - 1),
                        )
            for f in range(NF):
                for mc in range(NMC):
                    nc.vector.tensor_scalar_add(
                        kt_t[f][:, ts(mc, 512)],
                        ps_k[f * NMC + mc],
                        b2_t[:, f : f + 1],
                    )

            # ---- GEMM1: q2T[f,n] = sum_i WqT[i,f].T @ rnaT[i,n]  (+bq2) ----
            ps_q = [
                pspool.tile([128, 512], F32, tag=f"ps{j}", name=f"psq{j}")
                for j in range(8)
            ]
            for i in range(NQ):
                wqi = chunk_slice(WQ_CHUNKS, wq_c, i)
                for f in range(NF):
                    for nc_ in range(NMC):
                        nc.tensor.matmul(
                            ps_q[f * NMC + nc_],
                            wqi[:, ts(f, 128)],
                            xq_t[i][:, ts(nc_, 512)],
                            start=(i == 0),
                            stop=(i == NQ - 1),
                        )
            for f in range(NF):
                for nc_ in range(NMC):
                    nc.vector.tensor_scalar_add(
                        q2_t[f][:, ts(nc_, 512)],
                        ps_q[f * NMC + nc_],
                        b2_t[:, NF + f : NF + f + 1],
                    )

            # ---- GEMM3: out[n,m] = sum_f q2T[f,n].T @ kT[f,m] ----
            for nb in range(NB):
                for mc in range(NMC):
                    ps = pspool.tile(
                        [128, 512],
                        F32,
                        tag=f"ps{(nb * NMC + mc) % 8}",
                        name=f"ps3_{nb}_{mc}",
                    )
                    for f in range(NF):
                        nc.tensor.matmul(
                            ps,
                            q2_t[f][:, ts(nb, 128)],
                            kt_t[f][:, ts(mc, 512)],
                            start=(f == 0),
                            stop=(f == NF - 1),
                        )
                    if _LOOPMM and reps > 1:
                        continue  # timing probe: skip drain of GEMM3 psums
                    ot = opool.tile(
                        [128, 512], BF16, tag="ot", name=f"ot{nb}_{mc}"
                    )
                    if (nb + mc) % 2 == 0:
                        nc.vector.tensor_copy(ot, ps)
                    else:
                        nc.scalar.activation(
                            ot, ps, mybir.ActivationFunctionType.Copy
                        )
                    # stores on SWDGE (Pool) - third parallel DMA path
                    nc.gpsimd.dma_start(out_d[ts(nb, 128), ts(mc, 512)], ot)

    nc.compile()
    return nc


def _get_program(reps=1):
    key = ("nc", reps)
    if key not in _CACHE:
        _CACHE[key] = _build_program(reps)
    return _CACHE[key]


def _prep_inputs(rna_reps, protein_reps, Wq, bq, Wk, bk, rel_bias):
    bf16 = ml_dtypes.bfloat16
    # fold scale/H into Wq; fold rel_bias head-mean into the q bias
    rel_flat = np.asarray(rel_bias, np.float32).reshape(H * DK)
    wq2t = (np.asarray(Wq, np.float32).T * (SCALE / H)).astype(bf16)  # [DIM2,F]
    bq2 = (SCALE / H) * np.asarray(bq, np.float32) + rel_flat / H
    wkt = np.zeros((KINP, F), dtype=bf16)
    wkt[:KIN] = np.asarray(Wk, np.float32).T.astype(bf16)
    bk2 = np.asarray(bk, np.float32)

    # packed biases: col f -> bk chunk f, col NF+f -> bq chunk f
    b2 = np.empty((128, 2 * NF), np.float32)
    for f in range(NF):
        b2[:, f] = bk2[f * 128 : (f + 1) * 128]
        b2[:, NF + f] = bq2[f * 128 : (f + 1) * 128]

    if _HOSTT:
        # feature-major layout: [B, D, tokens]
        rna_bf = (
            np.asarray(rna_reps, np.float32)
            .transpose(0, 2, 1)
            .astype(bf16)
        )
        prot_bf = np.zeros((B, KINP, M), dtype=bf16)
        prot_bf[:, :KIN] = (
            np.asarray(protein_reps, np.float32)
            .transpose(0, 2, 1)
            .astype(bf16)
        )
    else:
        rna_bf = np.asarray(rna_reps, np.float32).astype(bf16)  # [B,N,DIM2]
        prot_bf = np.zeros((B, M, KINP), dtype=bf16)
        prot_bf[:, :, :KIN] = np.asarray(protein_reps, np.float32).astype(bf16)

    in_maps = []
    for b in range(B):
        in_maps.append(
            {
                "rna": np.ascontiguousarray(rna_bf[b]),
                "prot": np.ascontiguousarray(prot_bf[b]),
                "wqt": wq2t,
                "wkt": wkt,
                "b2": b2,
            }
        )
    return in_maps


def kernel(rna_reps, protein_reps, Wq, bq, Wk, bk, rel_bias, **_ignored):
    in_maps = _prep_inputs(rna_reps, protein_reps, Wq, bq, Wk, bk, rel_bias)
    nc = _get_program()
    res = bass_utils.run_bass_kernel_spmd(
        nc, in_maps, core_ids=list(range(NCORES))
    )
    out = np.stack(
        [np.asarray(res.results[b]["out"], np.float32) for b in range(B)], axis=0
    )
    return out


# revision 11
# speedup vs baseline: 1.1529x; 1.0638x over previous
"""CrossAttentionOutLayer Trainium2 kernel.

Math: reference computes, per batch b:
    q = rna @ Wq.T + bq                [n, h*dk]
    k = prot @ Wk.T + bk               [m, h*dk]
    logits[h] = (q_h*scale + rel_h) @ k_h.T
    out = mean_h logits                [n, m]

The head-mean of per-head inner products collapses into one flat inner
product over the h*dk=512 axis:
    out[i,j] = (scale/H * q[i,:] + rel_flat/H) . k[j,:]
so with Wq2 = (scale/H)*Wq, bq2 = (scale/H)*bq + rel_flat/H:
    out = (rna @ Wq2.T + bq2) @ (prot @ Wk.T + bk).T
Three GEMMs per batch. Data-parallel: batch b -> core b (8 cores).

On-device layout: feature-major ("transposed") activations via DMA x-bar
transpose (bf16), all GEMMs in bf16 with fp32 PSUM accumulation.
DMA issue is split across SP (transposes), ACT (weights), and GPSIMD
(output stores) so the PE's first matmul gates only on the first weight
chunk + first transposed tile.
"""

import os
from contextlib import ExitStack

import numpy as np
import ml_dtypes

# timing experiment: 1 = replace x-bar transposes with plain same-size DMAs
# (results become WRONG; only for isolating the transpose cost)
_NOTRANS = os.environ.get("KERNEL_NOTRANS", "0") == "1"
# 1 = activations shipped feature-major from the host (plain contiguous
# loads on device); 0 = natural layout + on-device x-bar DMA transpose
_HOSTT = os.environ.get("KERNEL_HOSTT", "1") == "1"
# 1 = (timing probe) repeat-loop wraps only the matmul/bias phases; DMAs
# and output copies/stores run once outside the loop
_LOOPMM = os.environ.get("KERNEL_LOOPMM", "0") == "1"
# 1 = (timing probe, with LOOPMM) every matmul uses the same lhsT slice
_FIXEDW = os.environ.get("KERNEL_FIXEDW", "0") == "1"

import concourse.bass as bass
import concourse.bacc as bacc
import concourse.tile as tile
import concourse.mybir as mybir
from concourse import bass_utils
from concourse.bass import ts

B, N, M = 8, 1024, 1024
DIM2 = 1280            # rna in-features  = 10*128
KIN = 1344             # protein in-features
KINP = 1408            # padded to 11*128
F = 512                # h*dk flat feature dim = 4*128
H, DK = 8, 64
SCALE = DK ** -0.5
NCORES = 8

NQ = DIM2 // 128       # 10 contraction tiles for Q gemm
NK = KINP // 128       # 11 contraction tiles for K gemm
NF = F // 128          # 4 feature tiles
NB = N // 128          # 8 row blocks of output
NMC = M // 512         # 2 column chunks of output

WK_CHUNKS = [4, 4, 3]  # contraction tiles per weight-load DMA
WQ_CHUNKS = [4, 4, 2]

BF16 = mybir.dt.bfloat16
F32 = mybir.dt.float32

_CACHE = {}


def _build_program(reps=1):
    nc = bacc.Bacc(
        "TRN2", target_bir_lowering=False, debug=False, num_devices=NCORES
    )

    if _HOSTT:
        rna_d = nc.dram_tensor("rna", [DIM2, N], BF16, kind="ExternalInput").ap()
        prot_d = nc.dram_tensor("prot", [KINP, M], BF16, kind="ExternalInput").ap()
    else:
        rna_d = nc.dram_tensor("rna", [N, DIM2], BF16, kind="ExternalInput").ap()
        prot_d = nc.dram_tensor("prot", [M, KINP], BF16, kind="ExternalInput").ap()
    wq_d = nc.dram_tensor("wqt", [DIM2, F], BF16, kind="ExternalInput").ap()
    wk_d = nc.dram_tensor("wkt", [KINP, F], BF16, kind="ExternalInput").ap()
    b2_d = nc.dram_tensor("b2", [128, 2 * NF], F32, kind="ExternalInput").ap()
    out_d = nc.dram_tensor("out", [N, M], BF16, kind="ExternalOutput").ap()

    with tile.TileContext(nc) as tc:
        with (
            tc.tile_pool(name="weights", bufs=1) as wpool,
            tc.tile_pool(name="acts", bufs=1) as apool,
            tc.tile_pool(name="qk", bufs=1) as qkpool,
            tc.tile_pool(name="bias", bufs=1) as bpool,
            tc.tile_pool(name="outs", bufs=4) as opool,
            tc.tile_pool(name="psum", bufs=1, space="PSUM") as pspool,
            ExitStack() as loop_ctx,
        ):
            if reps > 1 and not _LOOPMM:
                loop_ctx.enter_context(
                    tc.For_i(
                        0, reps, 1, hint_engines=(mybir.EngineType.PE,)
                    )
                )
            # ---- persistent SBUF tensors ----
            # weight chunk tiles: [128, n_ktiles_in_chunk, F]
            wk_c = [
                wpool.tile([128, sz, F], BF16, tag=f"wkc{j}", name=f"wkc{j}")
                for j, sz in enumerate(WK_CHUNKS)
            ]
            wq_c = [
                wpool.tile([128, sz, F], BF16, tag=f"wqc{j}", name=f"wqc{j}")
                for j, sz in enumerate(WQ_CHUNKS)
            ]

            def chunk_slice(chunks, tiles, i):
                """(chunk_tile, local_idx) for global contraction tile i."""
                if _FIXEDW:
                    return tiles[0][:, 0]
                j = 0
                while i >= chunks[j]:
                    i -= chunks[j]
                    j += 1
                return tiles[j][:, i]

            xk_t = [
                apool.tile([128, M], BF16, tag=f"xk{i}", name=f"xk{i}")
                for i in range(NK)
            ]
            xq_t = [
                apool.tile([128, N], BF16, tag=f"xq{i}", name=f"xq{i}")
                for i in range(NQ)
            ]
            kt_t = [
                qkpool.tile([128, M], BF16, tag=f"kt{f}", name=f"kt{f}")
                for f in range(NF)
            ]
            q2_t = [
                qkpool.tile([128, N], BF16, tag=f"q2{f}", name=f"q2{f}")
                for f in range(NF)
            ]
            b2_t = bpool.tile([128, 2 * NF], F32, tag="b2", name="b2sb")

            # ---- DMA issue plan ----
            # All input DMAs on SP (nc.sync), ordered exactly as the PE
            # consumes them: weight chunk j lands just before the first
            # matmul that needs it, transposes stream in between.
            def load_xpose(dst, src_d, i):
                if _HOSTT:
                    nc.sync.dma_start(dst, src_d[ts(i, 128), :])
                elif _NOTRANS:
                    nc.sync.dma_start(dst, src_d[0:128, 0 : dst.shape[1]])
                else:
                    nc.sync.dma_start(dst, src_d[:, ts(i, 128)], transpose=True)

            def load_wchunk(w_c, w_d, chunks, j):
                off = sum(chunks[:j])
                src = w_d[off * 128 : (off + chunks[j]) * 128, :]
                nc.sync.dma_start(
                    w_c[j], src.rearrange("(t p) f -> p t f", p=128)
                )

            load_wchunk(wk_c, wk_d, WK_CHUNKS, 0)
            for i in range(3):
                load_xpose(xk_t[i], prot_d, i)
            load_wchunk(wk_c, wk_d, WK_CHUNKS, 1)
            for i in range(3, 6):
                load_xpose(xk_t[i], prot_d, i)
            load_wchunk(wk_c, wk_d, WK_CHUNKS, 2)
            for i in range(6, NK):
                load_xpose(xk_t[i], prot_d, i)
            nc.sync.dma_start(b2_t, b2_d)
            for i in range(3):
                load_xpose(xq_t[i], rna_d, i)
            load_wchunk(wq_c, wq_d, WQ_CHUNKS, 0)
            for i in range(3, 6):
                load_xpose(xq_t[i], rna_d, i)
            load_wchunk(wq_c, wq_d, WQ_CHUNKS, 1)
            for i in range(6, NQ):
                load_xpose(xq_t[i], rna_d, i)
            load_wchunk(wq_c, wq_d, WQ_CHUNKS, 2)

            if reps > 1 and _LOOPMM:
                loop_ctx.enter_context(
                    tc.For_i(
                        0, reps, 1, hint_engines=(mybir.EngineType.PE,)
                    )
                )

            # ---- GEMM2: kT[f,m] = sum_i WkT[i,f].T @ protT[i,m]  (+bk) ----
            # contraction-outer so PE work starts as soon as xk[0] lands
            ps_k = [
                pspool.tile([128, 512], F32, tag=f"ps{j}", name=f"psk{j}")
                for j in range(8)
            ]
            for i in range(NK):
                wki = chunk_slice(WK_CHUNKS, wk_c, i)
                for f in range(NF):
                    for mc in range(NMC):
                        nc.tensor.matmul(
                            ps_k[f * NMC + mc],
                            wki[:, ts(f, 128)],
                            xk_t[i][:, ts(mc, 512)],
                            start=(i == 0),
                            stop=(i == NK - 1),
                        )
            for f in range(NF):
                for mc in range(NMC):
                    nc.vector.tensor_scalar_add(
                        kt_t[f][:, ts(mc, 512)],
                        ps_k[f * NMC + mc],
                        b2_t[:, f : f + 1],
                    )

            # ---- GEMM1: q2T[f,n] = sum_i WqT[i,f].T @ rnaT[i,n]  (+bq2) ----
            ps_q = [
                pspool.tile([128, 512], F32, tag=f"ps{j}", name=f"psq{j}")
                for j in range(8)
            ]
            for i in range(NQ):
                wqi = chunk_slice(WQ_CHUNKS, wq_c, i)
                for f in range(NF):
                    for nc_ in range(NMC):
                        nc.tensor.matmul(
                            ps_q[f * NMC + nc_],
                            wqi[:, ts(f, 128)],
                            xq_t[i][:, ts(nc_, 512)],
                            start=(i == 0),
                            stop=(i == NQ - 1),
                        )
            for f in range(NF):
                for nc_ in range(NMC):
                    nc.vector.tensor_scalar_add(
                        q2_t[f][:, ts(nc_, 512)],
                        ps_q[f * NMC + nc_],
                        b2_t[:, NF + f : NF + f + 1],
                    )

            # ---- GEMM3: out[n,m] = sum_f q2T[f,n].T @ kT[f,m] ----
            for nb in range(NB):
                for mc in range(NMC):
                    ps = pspool.tile(
                        [128, 512],
                        F32,
                        tag=f"ps{(nb * NMC + mc) % 8}",
                        name=f"ps3_{nb}_{mc}",
                    )
                    for f in range(NF):
                        nc.tensor.matmul(
                            ps,
                            q2_t[0][:, ts(0, 128)] if _FIXEDW
                            else q2_t[f][:, ts(nb, 128)],
                            kt_t[f][:, ts(mc, 512)],
                            start=(f == 0),
                            stop=(f == NF - 1),
                        )
                    if _LOOPMM and reps > 1:
                        continue  # timing probe: skip drain of GEMM3 psums
                    ot = opool.tile(
                        [128, 512], BF16, tag="ot", name=f"ot{nb}_{mc}"
                    )
                    if (nb + mc) % 2 == 0:
                        nc.vector.tensor_copy(ot, ps)
                    else:
                        nc.scalar.activation(
                            ot, ps, mybir.ActivationFunctionType.Copy
                        )
                    # stores on SWDGE (Pool) - third parallel DMA path
                    nc.gpsimd.dma_start(out_d[ts(nb, 128), ts(mc, 512)], ot)

    nc.compile()
    return nc


def _get_program(reps=1):
    key = ("nc", reps)
    if key not in _CACHE:
        _CACHE[key] = _build_program(reps)
    return _CACHE[key]


def _prep_inputs(rna_reps, protein_reps, Wq, bq, Wk, bk, rel_bias):
    bf16 = ml_dtypes.bfloat16
    # fold scale/H into Wq; fold rel_bias head-mean into the q bias
    rel_flat = np.asarray(rel_bias, np.float32).reshape(H * DK)
    wq2t = (np.asarray(Wq, np.float32).T * (SCALE / H)).astype(bf16)  # [DIM2,F]
    bq2 = (SCALE / H) * np.asarray(bq, np.float32) + rel_flat / H
    wkt = np.zeros((KINP, F), dtype=bf16)
    wkt[:KIN] = np.asarray(Wk, np.float32).T.astype(bf16)
    bk2 = np.asarray(bk, np.float32)

    # packed biases: col f -> bk chunk f, col NF+f -> bq chunk f
    b2 = np.empty((128, 2 * NF), np.float32)
    for f in range(NF):
        b2[:, f] = bk2[f * 128 : (f + 1) * 128]
        b2[:, NF + f] = bq2[f * 128 : (f + 1) * 128]

    if _HOSTT:
        # feature-major layout: [B, D, tokens]
        rna_bf = (
            np.asarray(rna_reps, np.float32)
            .transpose(0, 2, 1)
            .astype(bf16)
        )
        prot_bf = np.zeros((B, KINP, M), dtype=bf16)
        prot_bf[:, :KIN] = (
            np.asarray(protein_reps, np.float32)
            .transpose(0, 2, 1)
            .astype(bf16)
        )
    else:
        rna_bf = np.asarray(rna_reps, np.float32).astype(bf16)  # [B,N,DIM2]
        prot_bf = np.zeros((B, M, KINP), dtype=bf16)
        prot_bf[:, :, :KIN] = np.asarray(protein_reps, np.float32).astype(bf16)

    in_maps = []
    for b in range(B):
        in_maps.append(
            {
                "rna": np.ascontiguousarray(rna_bf[b]),
                "prot": np.ascontiguousarray(prot_bf[b]),
                "wqt": wq2t,
                "wkt": wkt,
                "b2": b2,
            }
        )
    return in_maps


def kernel(rna_reps, protein_reps, Wq, bq, Wk, bk, rel_bias, **_ignored):
    in_maps = _prep_inputs(rna_reps, protein_reps, Wq, bq, Wk, bk, rel_bias)
    nc = _get_program()
    res = bass_utils.run_bass_kernel_spmd(
        nc, in_maps, core_ids=list(range(NCORES))
    )
    out = np.stack(
        [np.asarray(res.results[b]["out"], np.float32) for b in range(B)], axis=0
    )
    return out
